# revision 33
# baseline (speedup 1.0000x reference)
"""Trainium2 Bass kernel for BuDingAttention (GQA attention layer).

Full inputs -> full output. Sharding: 8 cores = 2 batches x 4 KV heads.
Core c handles batch c//4, KV head c%4 (and its 4 query heads).
Each core computes q/k/v projections for its heads, RoPE, causal
attention, and the o_proj contribution of its heads (partial [S, H]).
Host sums the 4 partials per batch.

Numerics: fp16 matmul inputs (10-bit mantissa, same class as fp32r's
effective precision) with fp32 PSUM accumulation throughout.
Scores are computed transposed (ST[k, q] = K_chunk @ Q^T) so exp(ST)
feeds the PV matmul directly with no PE transposes. Softmax row sums
via ones-vector matmuls; 1/l applied to OT via a PE partition
broadcast of l and a 128-wide reciprocal.
"""
import numpy as np

import concourse.bass as bass
import concourse.tile as tile
from concourse import bacc, mybir
from concourse.bass import ts, ds
from concourse.bass_utils import run_bass_kernel_spmd

B, S, H = 2, 2048, 2048
NH, NKV, HD = 16, 4, 128
G = NH // NKV          # query heads per KV head (= heads per core)
P = 128                # partitions
SB = S // P            # 16 row blocks
NG = SB // 4           # 4 groups of 4 row blocks
SC = S // 512          # 4 col chunks of 512
KC = H // P            # 16 contraction chunks for projections
ROPE_THETA = 10000.0
NEG = -1e9
ISCALE = float(1.0 / np.sqrt(HD))

f32 = mybir.dt.float32
f32r = mybir.dt.float32r
f16 = mybir.dt.float16

_CACHE = {}
LAST_RESULTS = None  # for test.py introspection


def _build(mode: str):
    """mode: 'causal' | 'nomask' | 'mask'"""
    nc = bacc.Bacc("TRN2", target_bir_lowering=False, debug=False,
                   enable_asserts=False, num_devices=8)

    xt_d = nc.dram_tensor("xt", [H, S], f16, kind="ExternalInput").ap()
    wq_d = nc.dram_tensor("wq", [H, G * HD], f16, kind="ExternalInput").ap()
    wk_d = nc.dram_tensor("wk", [H, HD], f16, kind="ExternalInput").ap()
    wv_d = nc.dram_tensor("wv", [H, HD], f16, kind="ExternalInput").ap()
    wo_d = nc.dram_tensor("wo", [G * HD, H], f16, kind="ExternalInput").ap()
    bq_d = nc.dram_tensor("bq", [P, G], f32, kind="ExternalInput").ap()
    bk_d = nc.dram_tensor("bk", [P, 1], f32, kind="ExternalInput").ap()
    bv_d = nc.dram_tensor("bv", [P, 1], f32, kind="ExternalInput").ap()
    cost_d = nc.dram_tensor("cost", [P, S], f32, kind="ExternalInput").ap()
    sint_d = nc.dram_tensor("sint", [P, S], f32, kind="ExternalInput").ap()
    perm_d = nc.dram_tensor("perm", [P, P], f16, kind="ExternalInput").ap()
    idr_d = nc.dram_tensor("idr", [P, P], f16, kind="ExternalInput").ap()
    tri_d = nc.dram_tensor("tri", [P, P], f32, kind="ExternalInput").ap()
    onc_d = nc.dram_tensor("onc", [P, 1], f16, kind="ExternalInput").ap()
    onr_d = nc.dram_tensor("onr", [1, P], f32r, kind="ExternalInput").ap()
    tri01_d = nc.dram_tensor("tri01", [P, P], f16, kind="ExternalInput").ap()
    if mode == "mask":
        # transposed additive mask: maskT[k, q]
        mask_d = nc.dram_tensor("mask", [S, S], f32, kind="ExternalInput").ap()
    out_d = nc.dram_tensor("out", [S, H], f32, kind="ExternalOutput").ap()

    xt_r = xt_d.rearrange("(ko p) s -> p ko s", p=P)       # [128,16,2048]
    wq_r = wq_d.rearrange("(ko p) m -> p ko m", p=P)       # [128,16,512]
    wk_r = wk_d.rearrange("(ko p) m -> p ko m", p=P)       # [128,16,128]
    wv_r = wv_d.rearrange("(ko p) m -> p ko m", p=P)
    wo_r = wo_d.rearrange("(ho p) n -> p ho n", p=P)       # [128,4,2048]

    with tile.TileContext(nc) as tc:
        with (
            tc.tile_pool(name="consts", bufs=1) as consts,
            tc.tile_pool(name="weights", bufs=1) as wpool,
            tc.tile_pool(name="acts", bufs=1) as apool,
        ):
            perm_t = consts.tile([P, P], f16)
            idr_t = consts.tile([P, P], f16)
            tri_t = consts.tile([P, P], f32)
            bq_t = consts.tile([P, G], f32)
            bk_t = consts.tile([P, 1], f32)
            bv_t = consts.tile([P, 1], f32)
            onc_t = consts.tile([P, 1], f16)
            onr_t = consts.tile([1, P], f32r)
            tri01_t = consts.tile([P, P], f16)
            nc.sync.dma_start(tri01_t[:], tri01_d)
            nc.sync.dma_start(perm_t[:], perm_d)
            nc.sync.dma_start(idr_t[:], idr_d)
            nc.sync.dma_start(tri_t[:], tri_d)
            nc.sync.dma_start(bq_t[:], bq_d)
            nc.sync.dma_start(bk_t[:], bk_d)
            nc.sync.dma_start(bv_t[:], bv_d)
            nc.sync.dma_start(onc_t[:], onc_d)
            nc.sync.dma_start(onr_t[:], onr_d)

            wq_t = wpool.tile([P, KC, G * HD], f16)
            wk_t = wpool.tile([P, KC, HD], f16)
            wv_t = wpool.tile([P, KC, HD], f16)
            wo_t = wpool.tile([P, G, H], f16)
            # weights on the scalar DMA queue so xt/out keep sync free
            for i in range(4):
                nc.scalar.dma_start(wq_t[:, ts(i, 4), :],
                                    wq_r[:, ts(i, 4), :])
            nc.scalar.dma_start(wk_t[:], wk_r)
            nc.scalar.dma_start(wv_t[:], wv_r)
            for i in range(4):
                nc.scalar.dma_start(wo_t[:, i, :], wo_r[:, i, :])

            qt_t = apool.tile([P, G, S], f16)      # rotated Q^T per head
            kt_t = apool.tile([P, S], f16)         # rotated K^T
            v_t = apool.tile([P, SB, HD], f16)     # V rows, kpos on partition

            # ---------- Phase 1: QKV projections + RoPE ----------
            with (
                tc.tile_pool(name="ph1", bufs=1) as ph1,
                tc.tile_pool(name="ph1ps", bufs=1, space="PSUM") as ph1ps,
            ):
                cost_t = ph1.tile([P, S], f32)
                sint_t = ph1.tile([P, S], f32)
                nc.scalar.dma_start(cost_t[:], cost_d)
                nc.scalar.dma_start(sint_t[:], sint_d)

                for sc in range(SC):
                    scs = ds(sc * 512, 512)
                    xt_t = ph1.tile([P, KC, 512], f16, tag="xt", bufs=2)
                    for i in range(4):
                        nc.sync.dma_start(xt_t[:, ts(i, 4), :],
                                          xt_r[:, ts(i, 4), scs])
                    # blk 0..G-1: q blocks; G: k; G+1: v
                    for blk in range(G + 2):
                        ps = ph1ps.tile([P, 512], f32, tag="qkv_ps", bufs=3)
                        if blk < G:
                            w_ap = lambda kc: wq_t[:, kc, ts(blk, HD)]
                        elif blk == G:
                            w_ap = lambda kc: wk_t[:, kc, :]
                        else:
                            w_ap = lambda kc: wv_t[:, kc, :]
                        for kc in range(KC):
                            nc.tensor.matmul(ps[:], w_ap(kc), xt_t[:, kc, :],
                                             start=(kc == 0),
                                             stop=(kc == KC - 1))
                        if blk <= G:
                            # q/k: bias copyback then RoPE
                            bias = (bq_t[:, blk:blk + 1] if blk < G
                                    else bk_t[:, :])
                            raw = ph1.tile([P, 512], f16, tag="qkraw", bufs=3)
                            nc.scalar.activation(
                                raw[:], ps[:],
                                mybir.ActivationFunctionType.Identity,
                                bias=bias)
                            rot_ps = ph1ps.tile([P, 512], f32,
                                                tag="mps", bufs=2)
                            nc.tensor.matmul(rot_ps[:], perm_t[:], raw[:],
                                             start=True, stop=True)
                            tmp_sin = ph1.tile([P, 512], f32,
                                               tag="rtmp", bufs=4)
                            nc.vector.tensor_tensor(
                                tmp_sin[:], rot_ps[:], sint_t[:, scs],
                                mybir.AluOpType.mult)
                            tmp_cos = ph1.tile([P, 512], f32,
                                               tag="rtmp", bufs=4)
                            nc.vector.tensor_tensor(
                                tmp_cos[:], raw[:], cost_t[:, scs],
                                mybir.AluOpType.mult)
                            dst = (qt_t[:, blk, scs] if blk < G
                                   else kt_t[:, scs])
                            nc.vector.tensor_tensor(dst, tmp_cos[:],
                                                    tmp_sin[:],
                                                    mybir.AluOpType.add)
                        else:
                            # v: bias copyback then PE transpose to [s, hd]
                            vt_sb = ph1.tile([P, 512], f16, tag="vt", bufs=2)
                            nc.scalar.activation(
                                vt_sb[:], ps[:],
                                mybir.ActivationFunctionType.Identity,
                                bias=bv_t[:, :])
                            for j in range(4):
                                vtr_ps = ph1ps.tile([P, P], f16,
                                                    tag="mps", bufs=2)
                                nc.tensor.transpose(vtr_ps[:],
                                                    vt_sb[:, ts(j, P)],
                                                    idr_t[:])
                                nc.vector.tensor_copy(v_t[:, sc * 4 + j, :],
                                                      vtr_ps[:])

            # ---------- Phase 2: attention + o_proj ----------
            with (
                tc.tile_pool(name="ph2", bufs=1) as ph2,
                tc.tile_pool(name="ph2ps", bufs=1, space="PSUM") as ph2ps,
            ):
                if mode == "mask":
                    mrow = mask_d.rearrange("(kb p) s -> kb p s", p=P)
                for g in range(NG):
                    nkp = 4 * g + 4 if mode == "causal" else SB
                    ot_sb = ph2.tile([P, G, 512], f16, tag="OT", bufs=2)
                    for h in range(G):
                        ot_ps = ph2ps.tile([P, 512], f32, tag="ot_ps", bufs=2)
                        l_ps = ph2ps.tile([1, 512], f32, tag="op_ps", bufs=2)
                        # k chunks processed in pairs sharing one 1024-wide
                        # psum tile and a single exp
                        for kp in range(nkp // 2):
                            ks = [2 * kp, 2 * kp + 1]
                            s_ps = ph2ps.tile([P, 1024], f32,
                                              tag="sc_ps", bufs=2)
                            offs = []
                            for i, k in enumerate(ks):
                                off = (max(0, k - 4 * g) * P
                                       if mode == "causal" else 0)
                                offs.append(off)
                                w = 512 - off
                                nc.tensor.matmul(
                                    s_ps[:, ds(512 * i + off, w)],
                                    kt_t[:, ts(k, P)],
                                    qt_t[:, h, ds(512 * g + off, w)],
                                    start=True, stop=True)
                                if mode == "mask":
                                    m_sb = ph2.tile([P, 512], f32,
                                                    tag="msk", bufs=3)
                                    nc.sync.dma_start(
                                        m_sb[:, :w],
                                        mrow[k, :, ds(512 * g + off, w)])
                                    nc.vector.tensor_tensor(
                                        s_ps[:, ds(512 * i + off, w)],
                                        s_ps[:, ds(512 * i + off, w)],
                                        m_sb[:, :w], mybir.AluOpType.add)

                            se_t = ph2.tile([P, 1024], f16, tag="se", bufs=4)
                            # exp(s / sqrt(HD)); one pass when the pair's
                            # valid spans are uniform, else one per chunk
                            if offs[0] == offs[1]:
                                o0 = offs[0]
                                nc.scalar.activation(
                                    se_t[:, ds(o0, 1024 - o0)],
                                    s_ps[:, ds(o0, 1024 - o0)],
                                    mybir.ActivationFunctionType.Exp,
                                    scale=ISCALE)
                            else:
                                for i in range(2):
                                    oi = offs[i]
                                    nc.scalar.activation(
                                        se_t[:, ds(512 * i + oi, 512 - oi)],
                                        s_ps[:, ds(512 * i + oi, 512 - oi)],
                                        mybir.ActivationFunctionType.Exp,
                                        scale=ISCALE)
                            if mode == "causal":
                                # diag blocks: zero out k > q after the exp
                                # (off the PSUM path, on the idle GpSimd)
                                for i, k in enumerate(ks):
                                    if k >= 4 * g:
                                        dg = ds(512 * i + offs[i], P)
                                        nc.gpsimd.tensor_mul(
                                            se_t[:, dg], se_t[:, dg],
                                            tri01_t[:])
                            for i, k in enumerate(ks):
                                off = offs[i]
                                w = 512 - off
                                st = k == 0
                                sp = k == nkp - 1
                                nc.tensor.matmul(
                                    ot_ps[:, ds(off, w)],
                                    v_t[:, k, :],
                                    se_t[:, ds(512 * i + off, w)],
                                    start=st, stop=sp)
                                nc.tensor.matmul(
                                    l_ps[:, ds(off, w)],
                                    onc_t[:],
                                    se_t[:, ds(512 * i + off, w)],
                                    start=st, stop=sp)
                        # free ot_ps early: raw copy to SBUF, then the slow
                        # reciprocal chain only holds the rb slot
                        ot_raw = ph2.tile([P, 512], f32, tag="otr", bufs=2)
                        nc.scalar.copy(ot_raw[:], ot_ps[:])
                        # 1/l: broadcast l across partitions via PE, then
                        # 128-wide reciprocal, then scale OT
                        l_sb = ph2.tile([1, 512], f32r, tag="l_sb", bufs=2)
                        nc.scalar.copy(l_sb[:], l_ps[:])
                        rb_ps = ph2ps.tile([P, 512], f32, tag="ot_ps", bufs=2)
                        nc.tensor.matmul(rb_ps[:], onr_t[:], l_sb[:],
                                         start=True, stop=True)
                        rinv = ph2.tile([P, 512], f32, tag="rinv", bufs=2)
                        nc.vector.reciprocal(rinv[:], rb_ps[:])
                        nc.vector.tensor_tensor(
                            ot_sb[:, h, :], ot_raw[:], rinv[:],
                            mybir.AluOpType.mult)

                    # o_proj for this group's 4 row blocks
                    for j in range(4):
                        r = 4 * g + j
                        for hc in range(SC):
                            o_ps = ph2ps.tile([P, 512], f32,
                                              tag="op_ps", bufs=2)
                            for h in range(G):
                                nc.tensor.matmul(
                                    o_ps[:],
                                    ot_sb[:, h, ts(j, P)],
                                    wo_t[:, h, ts(hc, 512)],
                                    start=(h == 0), stop=(h == G - 1))
                            o_sb = ph2.tile([P, 512], f32, tag="osb", bufs=3)
                            if hc % 2 == 0:
                                nc.scalar.copy(o_sb[:], o_ps[:])
                            else:
                                nc.vector.tensor_copy(o_sb[:], o_ps[:])
                            nc.sync.dma_start(
                                out_d[ts(r, P), ts(hc, 512)], o_sb[:])

    nc.compile()
    return nc


def _get_program(mode: str):
    if mode not in _CACHE:
        _CACHE[mode] = _build(mode)
    return _CACHE[mode]


def _detect_mode(attention_mask: np.ndarray) -> str:
    m = attention_mask[:, 0]  # [B, S, S]
    if not np.isfinite(m).all():
        return "mask"
    if np.abs(m).max() == 0.0:
        return "nomask"
    iu = np.triu_indices(S, k=1)
    il = np.tril_indices(S, k=0)
    for b in range(m.shape[0]):
        if not (np.all(m[b][iu] <= -1e8) and np.all(m[b][il] == 0.0)):
            return "mask"
    return "causal"


def _rope_tables(position_ids: np.ndarray):
    """cos/sin transposed to [HD, S] per batch."""
    inv_freq = 1.0 / (ROPE_THETA **
                      (np.arange(0, HD, 2, dtype=np.float64) / HD))
    out = []
    for b in range(position_ids.shape[0]):
        freqs = position_ids[b].astype(np.float64)[:, None] * inv_freq
        emb = np.concatenate([freqs, freqs], axis=-1)  # [S, HD]
        cost = np.ascontiguousarray(np.cos(emb).T.astype(np.float32))
        sint = np.ascontiguousarray(np.sin(emb).T.astype(np.float32))
        out.append((cost, sint))
    return out


def kernel(hidden_states, wq, bq, wk, bk, wv, bv, wo,
           attention_mask, position_ids, _profile=False, _trace_cores=None):
    global LAST_RESULTS
    hidden_states = np.asarray(hidden_states, dtype=np.float32)
    wq = np.asarray(wq, dtype=np.float32)
    bq = np.asarray(bq, dtype=np.float32)
    wk = np.asarray(wk, dtype=np.float32)
    bk = np.asarray(bk, dtype=np.float32)
    wv = np.asarray(wv, dtype=np.float32)
    bv = np.asarray(bv, dtype=np.float32)
    wo = np.asarray(wo, dtype=np.float32)
    attention_mask = np.asarray(attention_mask, dtype=np.float32)
    position_ids = np.asarray(position_ids)

    mode = _detect_mode(attention_mask)
    nc = _get_program(mode)

    rope = _rope_tables(position_ids)

    half = HD // 2
    # lhsT for rot = Pi @ q: matmul computes lhsT.T @ rhs, Pi=[[0,-I],[I,0]]
    perm = np.zeros((P, P), dtype=np.float32)
    perm[0:half, half:P] = np.eye(half, dtype=np.float32)
    perm[half:P, 0:half] = -np.eye(half, dtype=np.float32)
    ident = np.eye(P, dtype=np.float32)
    # ST orientation [k, q]: mask out k > q (strictly-lower triangle)
    tri = np.where(np.triu(np.ones((P, P), dtype=bool)), 0.0, NEG)
    tri = tri.astype(np.float32)
    onc = np.ones((P, 1), dtype=np.float32)
    onr = np.ones((1, P), dtype=np.float32)
    # keep kpos <= q (ST orientation): upper triangle incl. diagonal
    tri01 = np.triu(np.ones((P, P), dtype=np.float32))

    # 1/sqrt(HD) applied as the exp's ACT scale parameter on-device
    in_maps = []
    for c in range(8):
        b, kv = c // NKV, c % NKV
        cost, sint = rope[b]
        f16i = {
            "xt": hidden_states[b].T,
            "wq": wq[:, 512 * kv:512 * (kv + 1)],
            "wk": wk[:, HD * kv:HD * (kv + 1)],
            "wv": wv[:, HD * kv:HD * (kv + 1)],
            "wo": wo[512 * kv:512 * (kv + 1), :],
            "perm": perm, "idr": ident, "onc": onc, "tri01": tri01,
        }
        f32i = {
            "bq": bq[512 * kv:512 * (kv + 1)].reshape(G, HD).T,
            "bk": bk[HD * kv:HD * (kv + 1)].reshape(HD, 1),
            "bv": bv[HD * kv:HD * (kv + 1)].reshape(HD, 1),
            "cost": cost, "sint": sint, "tri": tri, "onr": onr,
        }
        im = {k: np.ascontiguousarray(v, dtype=np.float16)
              for k, v in f16i.items()}
        im.update({k: np.ascontiguousarray(v, dtype=np.float32)
                   for k, v in f32i.items()})
        if mode == "mask":
            # transposed mask, pre-multiplied by sqrt(HD) since the exp
            # applies a 1/sqrt(HD) scale to (scores + mask)
            im["mask"] = np.ascontiguousarray(
                attention_mask[b, 0].T * np.sqrt(HD), dtype=np.float32)
        in_maps.append(im)

    kwargs = {}
    if _profile:
        kwargs = dict(trace=True,
                      trace_cores=_trace_cores or [0])
    res = run_bass_kernel_spmd(nc, in_maps, core_ids=list(range(8)), **kwargs)
    LAST_RESULTS = res

    out = np.zeros((B, S, H), dtype=np.float32)
    for c in range(8):
        out[c // NKV] += res.results[c]["out"]
    return out


# revision 35
# speedup vs baseline: 1.0669x; 1.0669x over previous
"""Trainium2 Bass kernel for BuDingAttention (GQA attention layer).

Full inputs -> full output. Sharding: 8 cores = 2 batches x 4 KV heads.
Core c handles batch c//4, KV head c%4 (and its 4 query heads).
Each core computes q/k/v projections for its heads, RoPE, causal
attention, and the o_proj contribution of its heads (partial [S, H]).
Host sums the 4 partials per batch.

Numerics: fp16 matmul inputs (10-bit mantissa, same class as fp32r's
effective precision) with fp32 PSUM accumulation throughout.
Scores are computed transposed (ST[k, q] = K_chunk @ Q^T) so exp(ST)
feeds the PV matmul directly with no PE transposes. Softmax row sums
via ones-vector matmuls; 1/l applied to OT via a PE partition
broadcast of l and a 128-wide reciprocal.
"""
import numpy as np

import concourse.bass as bass
import concourse.tile as tile
from concourse import bacc, mybir
from concourse.bass import ts, ds
from concourse.bass_utils import run_bass_kernel_spmd

B, S, H = 2, 2048, 2048
NH, NKV, HD = 16, 4, 128
G = NH // NKV          # query heads per KV head (= heads per core)
P = 128                # partitions
SB = S // P            # 16 row blocks
NG = SB // 4           # 4 groups of 4 row blocks
SC = S // 512          # 4 col chunks of 512
KC = H // P            # 16 contraction chunks for projections
ROPE_THETA = 10000.0
NEG = -1e9
ISCALE = float(1.0 / np.sqrt(HD))

f32 = mybir.dt.float32
f32r = mybir.dt.float32r
f16 = mybir.dt.float16

_CACHE = {}
LAST_RESULTS = None  # for test.py introspection


def _build(mode: str):
    """mode: 'causal' | 'nomask' | 'mask'"""
    nc = bacc.Bacc("TRN2", target_bir_lowering=False, debug=False,
                   enable_asserts=False, num_devices=8)

    xt_d = nc.dram_tensor("xt", [H, S], f16, kind="ExternalInput").ap()
    wq_d = nc.dram_tensor("wq", [H, G * HD], f16, kind="ExternalInput").ap()
    wk_d = nc.dram_tensor("wk", [H, HD], f16, kind="ExternalInput").ap()
    wv_d = nc.dram_tensor("wv", [H, HD], f16, kind="ExternalInput").ap()
    wo_d = nc.dram_tensor("wo", [G * HD, H], f16, kind="ExternalInput").ap()
    bq_d = nc.dram_tensor("bq", [P, G], f32, kind="ExternalInput").ap()
    bk_d = nc.dram_tensor("bk", [P, 1], f32, kind="ExternalInput").ap()
    bv_d = nc.dram_tensor("bv", [P, 1], f32, kind="ExternalInput").ap()
    cost_d = nc.dram_tensor("cost", [P, S], f32, kind="ExternalInput").ap()
    sint_d = nc.dram_tensor("sint", [P, S], f32, kind="ExternalInput").ap()
    perm_d = nc.dram_tensor("perm", [P, P], f16, kind="ExternalInput").ap()
    idr_d = nc.dram_tensor("idr", [P, P], f16, kind="ExternalInput").ap()
    tri_d = nc.dram_tensor("tri", [P, P], f32, kind="ExternalInput").ap()
    onc_d = nc.dram_tensor("onc", [P, 1], f16, kind="ExternalInput").ap()
    onr_d = nc.dram_tensor("onr", [1, P], f32r, kind="ExternalInput").ap()
    tri01_d = nc.dram_tensor("tri01", [P, P], f16, kind="ExternalInput").ap()
    if mode == "mask":
        # transposed additive mask: maskT[k, q]
        mask_d = nc.dram_tensor("mask", [S, S], f32, kind="ExternalInput").ap()
    out_d = nc.dram_tensor("out", [S, H], f32, kind="ExternalOutput").ap()

    xt_r = xt_d.rearrange("(ko p) s -> p ko s", p=P)       # [128,16,2048]
    wq_r = wq_d.rearrange("(ko p) m -> p ko m", p=P)       # [128,16,512]
    wk_r = wk_d.rearrange("(ko p) m -> p ko m", p=P)       # [128,16,128]
    wv_r = wv_d.rearrange("(ko p) m -> p ko m", p=P)
    wo_r = wo_d.rearrange("(ho p) n -> p ho n", p=P)       # [128,4,2048]

    with tile.TileContext(nc) as tc:
        with (
            tc.tile_pool(name="consts", bufs=1) as consts,
            tc.tile_pool(name="weights", bufs=1) as wpool,
            tc.tile_pool(name="acts", bufs=1) as apool,
        ):
            perm_t = consts.tile([P, P], f16)
            idr_t = consts.tile([P, P], f16)
            tri_t = consts.tile([P, P], f32)
            bq_t = consts.tile([P, G], f32)
            bk_t = consts.tile([P, 1], f32)
            bv_t = consts.tile([P, 1], f32)
            onc_t = consts.tile([P, 1], f16)
            onr_t = consts.tile([1, P], f32r)
            tri01_t = consts.tile([P, P], f16)
            nc.sync.dma_start(tri01_t[:], tri01_d)
            nc.sync.dma_start(perm_t[:], perm_d)
            nc.sync.dma_start(idr_t[:], idr_d)
            nc.sync.dma_start(tri_t[:], tri_d)
            nc.sync.dma_start(bq_t[:], bq_d)
            nc.sync.dma_start(bk_t[:], bk_d)
            nc.sync.dma_start(bv_t[:], bv_d)
            nc.sync.dma_start(onc_t[:], onc_d)
            nc.sync.dma_start(onr_t[:], onr_d)

            wq_t = wpool.tile([P, KC, G * HD], f16)
            wk_t = wpool.tile([P, KC, HD], f16)
            wv_t = wpool.tile([P, KC, HD], f16)
            wo_t = wpool.tile([P, G, H], f16)
            # weights on the scalar DMA queue so xt/out keep sync free
            for i in range(4):
                nc.scalar.dma_start(wq_t[:, ts(i, 4), :],
                                    wq_r[:, ts(i, 4), :])
            nc.scalar.dma_start(wk_t[:], wk_r)
            nc.scalar.dma_start(wv_t[:], wv_r)
            for i in range(4):
                nc.scalar.dma_start(wo_t[:, i, :], wo_r[:, i, :])

            qt_t = apool.tile([P, G, S], f16)      # rotated Q^T per head
            kt_t = apool.tile([P, S], f16)         # rotated K^T
            v_t = apool.tile([P, SB, HD], f16)     # V rows, kpos on partition

            # ---------- Phase 1: QKV projections + RoPE ----------
            with (
                tc.tile_pool(name="ph1", bufs=1) as ph1,
                tc.tile_pool(name="ph1ps", bufs=1, space="PSUM") as ph1ps,
            ):
                cost_t = ph1.tile([P, S], f32)
                sint_t = ph1.tile([P, S], f32)
                nc.scalar.dma_start(cost_t[:], cost_d)
                nc.scalar.dma_start(sint_t[:], sint_d)

                for sc in range(SC):
                    scs = ds(sc * 512, 512)
                    xt_t = ph1.tile([P, KC, 512], f16, tag="xt", bufs=2)
                    for i in range(4):
                        nc.sync.dma_start(xt_t[:, ts(i, 4), :],
                                          xt_r[:, ts(i, 4), scs])
                    # blk 0..G-1: q blocks; G: k; G+1: v
                    for blk in range(G + 2):
                        ps = ph1ps.tile([P, 512], f32, tag="qkv_ps", bufs=3)
                        if blk < G:
                            w_ap = lambda kc: wq_t[:, kc, ts(blk, HD)]
                        elif blk == G:
                            w_ap = lambda kc: wk_t[:, kc, :]
                        else:
                            w_ap = lambda kc: wv_t[:, kc, :]
                        for kc in range(KC):
                            nc.tensor.matmul(ps[:], w_ap(kc), xt_t[:, kc, :],
                                             start=(kc == 0),
                                             stop=(kc == KC - 1))
                        if blk <= G:
                            # q/k: bias copyback then RoPE
                            bias = (bq_t[:, blk:blk + 1] if blk < G
                                    else bk_t[:, :])
                            raw = ph1.tile([P, 512], f16, tag="qkraw", bufs=3)
                            nc.scalar.activation(
                                raw[:], ps[:],
                                mybir.ActivationFunctionType.Identity,
                                bias=bias)
                            rot_ps = ph1ps.tile([P, 512], f32,
                                                tag="mps", bufs=2)
                            nc.tensor.matmul(rot_ps[:], perm_t[:], raw[:],
                                             start=True, stop=True)
                            tmp_sin = ph1.tile([P, 512], f32,
                                               tag="rtmp", bufs=4)
                            nc.vector.tensor_tensor(
                                tmp_sin[:], rot_ps[:], sint_t[:, scs],
                                mybir.AluOpType.mult)
                            tmp_cos = ph1.tile([P, 512], f32,
                                               tag="rtmp", bufs=4)
                            nc.vector.tensor_tensor(
                                tmp_cos[:], raw[:], cost_t[:, scs],
                                mybir.AluOpType.mult)
                            dst = (qt_t[:, blk, scs] if blk < G
                                   else kt_t[:, scs])
                            nc.vector.tensor_tensor(dst, tmp_cos[:],
                                                    tmp_sin[:],
                                                    mybir.AluOpType.add)
                        else:
                            # v: bias copyback then PE transpose to [s, hd]
                            vt_sb = ph1.tile([P, 512], f16, tag="vt", bufs=2)
                            nc.scalar.activation(
                                vt_sb[:], ps[:],
                                mybir.ActivationFunctionType.Identity,
                                bias=bv_t[:, :])
                            for j in range(4):
                                vtr_ps = ph1ps.tile([P, P], f16,
                                                    tag="mps", bufs=2)
                                nc.tensor.transpose(vtr_ps[:],
                                                    vt_sb[:, ts(j, P)],
                                                    idr_t[:])
                                nc.vector.tensor_copy(v_t[:, sc * 4 + j, :],
                                                      vtr_ps[:])

            # ---------- Phase 2: attention + o_proj ----------
            with (
                tc.tile_pool(name="ph2", bufs=1) as ph2,
                tc.tile_pool(name="ph2ps", bufs=1, space="PSUM") as ph2ps,
            ):
                if mode == "mask":
                    mrow = mask_d.rearrange("(kb p) s -> kb p s", p=P)

                def oproj_block(gp, j, ot_tile):
                    """o_proj for row block 4*gp+j — also serves as PE
                    filler work at the next group's head boundaries."""
                    r = 4 * gp + j
                    for hc in range(SC):
                        o_ps = ph2ps.tile([P, 512], f32,
                                          tag="op_ps", bufs=2, name="o_ps")
                        for h in range(G):
                            nc.tensor.matmul(
                                o_ps[:],
                                ot_tile[:, h, ts(j, P)],
                                wo_t[:, h, ts(hc, 512)],
                                start=(h == 0), stop=(h == G - 1))
                        o_sb = ph2.tile([P, 512], f32, tag="osb", bufs=3,
                                        name="o_sb")
                        if hc % 2 == 0:
                            nc.scalar.copy(o_sb[:], o_ps[:])
                        else:
                            nc.vector.tensor_copy(o_sb[:], o_ps[:])
                        nc.sync.dma_start(
                            out_d[ts(r, P), ts(hc, 512)], o_sb[:])

                ot_prev = None
                for g in range(NG):
                    nkp = 4 * g + 4 if mode == "causal" else SB
                    ot_sb = ph2.tile([P, G, 512], f16, tag="OT", bufs=2)
                    for h in range(G):
                        if ot_prev is not None:
                            oproj_block(g - 1, h, ot_prev)
                        ot_ps = ph2ps.tile([P, 512], f32, tag="ot_ps", bufs=2)
                        l_ps = ph2ps.tile([1, 512], f32, tag="op_ps", bufs=2)
                        # k chunks processed in pairs sharing one 1024-wide
                        # psum tile and a single exp
                        for kp in range(nkp // 2):
                            ks = [2 * kp, 2 * kp + 1]
                            s_ps = ph2ps.tile([P, 1024], f32,
                                              tag="sc_ps", bufs=2)
                            offs = []
                            for i, k in enumerate(ks):
                                off = (max(0, k - 4 * g) * P
                                       if mode == "causal" else 0)
                                offs.append(off)
                                w = 512 - off
                                nc.tensor.matmul(
                                    s_ps[:, ds(512 * i + off, w)],
                                    kt_t[:, ts(k, P)],
                                    qt_t[:, h, ds(512 * g + off, w)],
                                    start=True, stop=True)
                                if mode == "mask":
                                    m_sb = ph2.tile([P, 512], f32,
                                                    tag="msk", bufs=3)
                                    nc.sync.dma_start(
                                        m_sb[:, :w],
                                        mrow[k, :, ds(512 * g + off, w)])
                                    nc.vector.tensor_tensor(
                                        s_ps[:, ds(512 * i + off, w)],
                                        s_ps[:, ds(512 * i + off, w)],
                                        m_sb[:, :w], mybir.AluOpType.add)

                            se_t = ph2.tile([P, 1024], f16, tag="se", bufs=4)
                            # exp(s / sqrt(HD)); one pass when the pair's
                            # valid spans are uniform, else one per chunk
                            if offs[0] == offs[1]:
                                o0 = offs[0]
                                nc.scalar.activation(
                                    se_t[:, ds(o0, 1024 - o0)],
                                    s_ps[:, ds(o0, 1024 - o0)],
                                    mybir.ActivationFunctionType.Exp,
                                    scale=ISCALE)
                            else:
                                for i in range(2):
                                    oi = offs[i]
                                    nc.scalar.activation(
                                        se_t[:, ds(512 * i + oi, 512 - oi)],
                                        s_ps[:, ds(512 * i + oi, 512 - oi)],
                                        mybir.ActivationFunctionType.Exp,
                                        scale=ISCALE)
                            if mode == "causal":
                                # diag blocks: zero out k > q after the exp
                                # (off the PSUM path, on the idle GpSimd)
                                for i, k in enumerate(ks):
                                    if k >= 4 * g:
                                        dg = ds(512 * i + offs[i], P)
                                        nc.gpsimd.tensor_mul(
                                            se_t[:, dg], se_t[:, dg],
                                            tri01_t[:])
                            for i, k in enumerate(ks):
                                off = offs[i]
                                w = 512 - off
                                st = k == 0
                                sp = k == nkp - 1
                                nc.tensor.matmul(
                                    ot_ps[:, ds(off, w)],
                                    v_t[:, k, :],
                                    se_t[:, ds(512 * i + off, w)],
                                    start=st, stop=sp)
                                nc.tensor.matmul(
                                    l_ps[:, ds(off, w)],
                                    onc_t[:],
                                    se_t[:, ds(512 * i + off, w)],
                                    start=st, stop=sp)
                        # free ot_ps early: raw copy to SBUF, then the slow
                        # reciprocal chain only holds the rb slot
                        ot_raw = ph2.tile([P, 512], f32, tag="otr", bufs=2)
                        nc.scalar.copy(ot_raw[:], ot_ps[:])
                        # 1/l: broadcast l across partitions via PE, then
                        # 128-wide reciprocal, then scale OT
                        l_sb = ph2.tile([1, 512], f32r, tag="l_sb", bufs=2)
                        nc.scalar.copy(l_sb[:], l_ps[:])
                        rb_ps = ph2ps.tile([P, 512], f32, tag="ot_ps", bufs=2)
                        nc.tensor.matmul(rb_ps[:], onr_t[:], l_sb[:],
                                         start=True, stop=True)
                        rinv = ph2.tile([P, 512], f32, tag="rinv", bufs=2)
                        nc.vector.reciprocal(rinv[:], rb_ps[:])
                        nc.vector.tensor_tensor(
                            ot_sb[:, h, :], ot_raw[:], rinv[:],
                            mybir.AluOpType.mult)
                    ot_prev = ot_sb

                # final group's o_proj
                for j in range(4):
                    oproj_block(NG - 1, j, ot_prev)

    nc.compile()
    return nc


def _get_program(mode: str):
    if mode not in _CACHE:
        _CACHE[mode] = _build(mode)
    return _CACHE[mode]


def _detect_mode(attention_mask: np.ndarray) -> str:
    m = attention_mask[:, 0]  # [B, S, S]
    if not np.isfinite(m).all():
        return "mask"
    if np.abs(m).max() == 0.0:
        return "nomask"
    iu = np.triu_indices(S, k=1)
    il = np.tril_indices(S, k=0)
    for b in range(m.shape[0]):
        if not (np.all(m[b][iu] <= -1e8) and np.all(m[b][il] == 0.0)):
            return "mask"
    return "causal"


def _rope_tables(position_ids: np.ndarray):
    """cos/sin transposed to [HD, S] per batch."""
    inv_freq = 1.0 / (ROPE_THETA **
                      (np.arange(0, HD, 2, dtype=np.float64) / HD))
    out = []
    for b in range(position_ids.shape[0]):
        freqs = position_ids[b].astype(np.float64)[:, None] * inv_freq
        emb = np.concatenate([freqs, freqs], axis=-1)  # [S, HD]
        cost = np.ascontiguousarray(np.cos(emb).T.astype(np.float32))
        sint = np.ascontiguousarray(np.sin(emb).T.astype(np.float32))
        out.append((cost, sint))
    return out


def kernel(hidden_states, wq, bq, wk, bk, wv, bv, wo,
           attention_mask, position_ids, _profile=False, _trace_cores=None):
    global LAST_RESULTS
    hidden_states = np.asarray(hidden_states, dtype=np.float32)
    wq = np.asarray(wq, dtype=np.float32)
    bq = np.asarray(bq, dtype=np.float32)
    wk = np.asarray(wk, dtype=np.float32)
    bk = np.asarray(bk, dtype=np.float32)
    wv = np.asarray(wv, dtype=np.float32)
    bv = np.asarray(bv, dtype=np.float32)
    wo = np.asarray(wo, dtype=np.float32)
    attention_mask = np.asarray(attention_mask, dtype=np.float32)
    position_ids = np.asarray(position_ids)

    mode = _detect_mode(attention_mask)
    nc = _get_program(mode)

    rope = _rope_tables(position_ids)

    half = HD // 2
    # lhsT for rot = Pi @ q: matmul computes lhsT.T @ rhs, Pi=[[0,-I],[I,0]]
    perm = np.zeros((P, P), dtype=np.float32)
    perm[0:half, half:P] = np.eye(half, dtype=np.float32)
    perm[half:P, 0:half] = -np.eye(half, dtype=np.float32)
    ident = np.eye(P, dtype=np.float32)
    # ST orientation [k, q]: mask out k > q (strictly-lower triangle)
    tri = np.where(np.triu(np.ones((P, P), dtype=bool)), 0.0, NEG)
    tri = tri.astype(np.float32)
    onc = np.ones((P, 1), dtype=np.float32)
    onr = np.ones((1, P), dtype=np.float32)
    # keep kpos <= q (ST orientation): upper triangle incl. diagonal
    tri01 = np.triu(np.ones((P, P), dtype=np.float32))

    # 1/sqrt(HD) applied as the exp's ACT scale parameter on-device
    in_maps = []
    for c in range(8):
        b, kv = c // NKV, c % NKV
        cost, sint = rope[b]
        f16i = {
            "xt": hidden_states[b].T,
            "wq": wq[:, 512 * kv:512 * (kv + 1)],
            "wk": wk[:, HD * kv:HD * (kv + 1)],
            "wv": wv[:, HD * kv:HD * (kv + 1)],
            "wo": wo[512 * kv:512 * (kv + 1), :],
            "perm": perm, "idr": ident, "onc": onc, "tri01": tri01,
        }
        f32i = {
            "bq": bq[512 * kv:512 * (kv + 1)].reshape(G, HD).T,
            "bk": bk[HD * kv:HD * (kv + 1)].reshape(HD, 1),
            "bv": bv[HD * kv:HD * (kv + 1)].reshape(HD, 1),
            "cost": cost, "sint": sint, "tri": tri, "onr": onr,
        }
        im = {k: np.ascontiguousarray(v, dtype=np.float16)
              for k, v in f16i.items()}
        im.update({k: np.ascontiguousarray(v, dtype=np.float32)
                   for k, v in f32i.items()})
        if mode == "mask":
            # transposed mask, pre-multiplied by sqrt(HD) since the exp
            # applies a 1/sqrt(HD) scale to (scores + mask)
            im["mask"] = np.ascontiguousarray(
                attention_mask[b, 0].T * np.sqrt(HD), dtype=np.float32)
        in_maps.append(im)

    kwargs = {}
    if _profile:
        kwargs = dict(trace=True,
                      trace_cores=_trace_cores or [0])
    res = run_bass_kernel_spmd(nc, in_maps, core_ids=list(range(8)), **kwargs)
    LAST_RESULTS = res

    out = np.zeros((B, S, H), dtype=np.float32)
    for c in range(8):
        out[c // NKV] += res.results[c]["out"]
    return out
